# revision 3
# baseline (speedup 1.0000x reference)
"""Trainium2 Bass kernel for nn_SingleConv (gnn_message_passing).

Computes, for each edge e:
  h  = relu(LN(f @ w1.T + b1)); h = relu(LN(h @ w2.T + b2))
  r  = h @ w3.T + b3                      # [E, co*ci*nf]
  out[e, co, do, ci, di] = sum_f r[e, co, ci, f] * basis[e, do, di, f]
returned as [E, 96, 96] fp32.

Sharding: pure data-parallel over E across 8 NeuronCores (2500 edges each).

Per-core kernel structure (128-edge tiles):
  - PE transpose + fp32 matmuls for the tiny MLP; LayerNorm via bn_stats
    with the normalize+relu fused into one ScalarE activation.
  - r = h2 @ w3T in fp16 on PE (w3 host-permuted to [h, (f, co, ci)]).
  - basis contraction as diagonal-matrix matmuls on PE: for each
    (do,di) pair, out_dd[e, co*ci] = sum_f diag(basis[:,do,di,f]) @ r_f,
    accumulated in PSUM over f.
  - PSUM -> SBUF evacuation scatters (co,ci) into the final
    (co*3+do)*96 + ci*3+di layout so the output DMA is contiguous.
"""

import sys

for _p in ("/opt/trn_rl_repo", "/root/.axon_site/_ro/trn_rl_repo"):
    if _p not in sys.path:
        sys.path.insert(0, _p)

import numpy as np

import concourse.bass as bass
import concourse.bacc as bacc
import concourse.tile as tile
from concourse import mybir
from concourse.bass_utils import run_bass_kernel_spmd

E = 20000
N_CORES = 8
EC = E // N_CORES  # 2500 edges per core
P = 128
F_IN = 17  # edge_dim + 1
CH = 32
NF = 3
D = 3  # d_out == d_in == 3
RW = CH * CH  # 1024, free width of one f-slice of r
OUTW = 96 * 96  # 9216
EPS = 1e-5

AF = mybir.ActivationFunctionType
ALU = mybir.AluOpType
dt = mybir.dt


def _layernorm_fused(nc, tc, pools, ps_x, e, out_tile, out_slice):
    """LN over the free dim (32) of psum tile ps_x[:e, :32], fused with relu,
    writing to out_tile[out_slice]. Fast path (gamma==1, beta==0)."""
    stats = pools["stat"].tile([P, 6], dt.float32, tag="stats")
    nc.vector.bn_stats(stats[:e], ps_x[:e])
    mv = pools["stat"].tile([P, 2], dt.float32, tag="mv")
    nc.vector.bn_aggr(mv[:e], stats[:e])
    std = pools["stat"].tile([P, 1], dt.float32, tag="std")
    nc.scalar.activation(std[:e], mv[:e, 1:2], AF.Sqrt, bias=pools["eps"][:e])
    rstd = pools["stat"].tile([P, 1], dt.float32, tag="rstd")
    nc.vector.reciprocal(rstd[:e], std[:e])
    # nmr = -mu * rstd
    nmr = pools["stat"].tile([P, 1], dt.float32, tag="nmr")
    nc.vector.tensor_scalar(nmr[:e], mv[:e, 0:1], rstd[:e], -1.0, ALU.mult, ALU.mult)
    # out = relu(x * rstd - mu * rstd)
    nc.scalar.activation(out_tile[out_slice], ps_x[:e], AF.Relu, bias=nmr[:e], scale=rstd[:e])


def build_program(n_edges):
    """Build the per-core Bass program; returns (nc, input_names)."""
    nc = bacc.Bacc("TRN2", target_bir_lowering=False, debug=False, num_devices=N_CORES)

    f_d = nc.dram_tensor("f", [n_edges, F_IN], dt.float32, kind="ExternalInput").ap()
    basis_d = nc.dram_tensor("basis27", [n_edges, 27], dt.float32, kind="ExternalInput").ap()
    w1t_d = nc.dram_tensor("w1t", [F_IN, CH], dt.float32, kind="ExternalInput").ap()
    w2t_d = nc.dram_tensor("w2t", [CH, CH], dt.float32, kind="ExternalInput").ap()
    w3t_d = nc.dram_tensor("w3t", [CH, NF * RW], dt.float16, kind="ExternalInput").ap()
    id32_d = nc.dram_tensor("ident32", [P, P], dt.float32, kind="ExternalInput").ap()
    id16_d = nc.dram_tensor("ident16", [P, P], dt.float16, kind="ExternalInput").ap()
    out_d = nc.dram_tensor("out", [n_edges, OUTW], dt.float32, kind="ExternalOutput").ap()

    n_tiles = (n_edges + P - 1) // P

    with tile.TileContext(nc) as tc:
        import contextlib

        with contextlib.ExitStack() as ctx:
            consts = ctx.enter_context(tc.tile_pool(name="consts", bufs=1))
            io_pool = ctx.enter_context(tc.tile_pool(name="io", bufs=3))
            mlp_pool = ctx.enter_context(tc.tile_pool(name="mlp", bufs=2))
            stat_pool = ctx.enter_context(tc.tile_pool(name="stat", bufs=2))
            r_pool = ctx.enter_context(tc.tile_pool(name="r", bufs=2))
            diag_pool = ctx.enter_context(tc.tile_pool(name="diag", bufs=4))
            out_pool = ctx.enter_context(tc.tile_pool(name="outp", bufs=2))
            ps_small = ctx.enter_context(tc.tile_pool(name="ps_small", bufs=2, space="PSUM"))
            ps_r = ctx.enter_context(tc.tile_pool(name="ps_r", bufs=2, space="PSUM"))
            ps_out = ctx.enter_context(tc.tile_pool(name="ps_out", bufs=2, space="PSUM"))

            pools = {"stat": stat_pool}

            # --- constants (loaded once) ---
            w1t_sb = consts.tile([F_IN, CH], dt.float32)
            nc.sync.dma_start(w1t_sb[:], w1t_d[:])
            w2t_sb = consts.tile([CH, CH], dt.float32)
            nc.sync.dma_start(w2t_sb[:], w2t_d[:])
            w3t_sb = consts.tile([CH, NF * RW], dt.float16)
            nc.sync.dma_start(w3t_sb[:], w3t_d[:])
            id32_sb = consts.tile([P, P], dt.float32)
            nc.sync.dma_start(id32_sb[:], id32_d[:])
            id16_sb = consts.tile([P, P], dt.float16)
            nc.sync.dma_start(id16_sb[:], id16_d[:])
            eps_sb = consts.tile([P, 1], dt.float32)
            nc.vector.memset(eps_sb[:], EPS)
            pools["eps"] = eps_sb

            for it in range(n_tiles):
                e0 = it * P
                e = min(P, n_edges - e0)

                f_sb = io_pool.tile([P, F_IN], dt.float32, tag="f")
                nc.sync.dma_start(f_sb[:e], f_d[e0 : e0 + e])
                b_sb = io_pool.tile([P, 27], dt.float32, tag="b")
                nc.sync.dma_start(b_sb[:e], basis_d[e0 : e0 + e])

                # --- fT via PE transpose ---
                ps_ft = ps_small.tile([F_IN, P], dt.float32, tag="pss")
                nc.tensor.transpose(ps_ft[:, :e], f_sb[:e, :], id32_sb[:e, :e])
                ft_sb = mlp_pool.tile([F_IN, P], dt.float32, tag="ft")
                nc.scalar.activation(ft_sb[:, :e], ps_ft[:, :e], AF.Copy)

                # --- layer 1 ---
                ps_h1 = ps_small.tile([P, CH], dt.float32, tag="pss")
                nc.tensor.matmul(ps_h1[:e], ft_sb[:, :e], w1t_sb[:], start=True, stop=True)
                h1n = mlp_pool.tile([P, CH], dt.float32, tag="h1n")
                _layernorm_fused(nc, tc, pools, ps_h1, e, h1n, np.s_[:e])

                # --- layer 2 ---
                ps_t1 = ps_small.tile([CH, P], dt.float32, tag="pss")
                nc.tensor.transpose(ps_t1[:, :e], h1n[:e, :], id32_sb[:e, :e])
                h1nT = mlp_pool.tile([CH, P], dt.float32, tag="h1nT")
                nc.scalar.activation(h1nT[:, :e], ps_t1[:, :e], AF.Copy)
                ps_h2 = ps_small.tile([P, CH], dt.float32, tag="pss")
                nc.tensor.matmul(ps_h2[:e], h1nT[:, :e], w2t_sb[:], start=True, stop=True)
                h2n = mlp_pool.tile([P, CH], dt.float32, tag="h2n")
                _layernorm_fused(nc, tc, pools, ps_h2, e, h2n, np.s_[:e])

                # --- h2nT in fp16 ---
                ps_t2 = ps_small.tile([CH, P], dt.float32, tag="pss")
                nc.tensor.transpose(ps_t2[:, :e], h2n[:e, :], id32_sb[:e, :e])
                h2nT = mlp_pool.tile([CH, P], dt.float16, tag="h2nT")
                nc.scalar.activation(h2nT[:, :e], ps_t2[:, :e], AF.Copy)

                # --- r = h2 @ w3T (fp16), f-major layout [f, co, ci] ---
                r_sb = r_pool.tile([P, NF * RW], dt.float16, tag="r")
                for fi in range(NF):
                    for j in range(2):
                        c0 = fi * RW + j * 512
                        ps_rr = ps_r.tile([P, 512], dt.float32, tag="psr")
                        nc.tensor.matmul(
                            ps_rr[:e], h2nT[:, :e], w3t_sb[:, c0 : c0 + 512],
                            start=True, stop=True,
                        )
                        nc.scalar.activation(r_sb[:e, c0 : c0 + 512], ps_rr[:e], AF.Copy)

                # --- basis contraction: 9 (do,di) pairs, accumulate over f ---
                out_sb = out_pool.tile([P, OUTW], dt.float32, tag="out")
                out_v = out_sb.rearrange(
                    "p (co d ci q) -> p co d ci q", co=CH, d=D, ci=CH, q=D
                )
                for dd in range(D * D):
                    do_, di_ = divmod(dd, D)
                    ps_o = ps_out.tile([P, RW], dt.float32, tag="pso")
                    for fi in range(NF):
                        dg = diag_pool.tile([P, P], dt.float16, tag="dg")
                        nc.vector.tensor_scalar_mul(
                            dg[:e, :e], id16_sb[:e, :e],
                            b_sb[:e, dd * NF + fi : dd * NF + fi + 1],
                        )
                        for j in range(2):
                            nc.tensor.matmul(
                                ps_o[:e, j * 512 : (j + 1) * 512],
                                dg[:e, :e],
                                r_sb[:e, fi * RW + j * 512 : fi * RW + (j + 1) * 512],
                                start=(fi == 0), stop=(fi == NF - 1),
                            )
                    src = ps_o[:e].rearrange("p (co ci) -> p co ci", co=CH)
                    dst = out_v[:e, :, do_, :, di_]
                    if dd % 3 == 2:
                        nc.scalar.activation(dst, src, AF.Copy)
                    else:
                        nc.vector.tensor_copy(dst, src)

                # --- store ---
                for k in range(4):
                    c0 = k * (OUTW // 4)
                    nc.sync.dma_start(
                        out_d[e0 : e0 + e, c0 : c0 + OUTW // 4],
                        out_sb[:e, c0 : c0 + OUTW // 4],
                    )

    nc.compile()
    return nc


_CACHE = {}


def _get_program(n_edges):
    if n_edges not in _CACHE:
        _CACHE[n_edges] = build_program(n_edges)
    return _CACHE[n_edges]


def prepare_host_inputs(f, basis, w1, b1, g1, be1, w2, b2, g2, be2, w3, b3):
    """Host-side prep: fold trivial affine/bias params, permute w3, build
    per-core input maps. Only the fast path (zero biases, unit gains) is
    supported on-device; non-trivial params are folded on the host where
    mathematically exact, else rejected."""
    f = np.asarray(f, np.float32)
    basis = np.asarray(basis, np.float32).reshape(E, 27)
    w1 = np.asarray(w1, np.float32)
    w2 = np.asarray(w2, np.float32)
    w3 = np.asarray(w3, np.float32)
    b1 = np.asarray(b1, np.float32)
    b2 = np.asarray(b2, np.float32)
    b3 = np.asarray(b3, np.float32)
    g1 = np.asarray(g1, np.float32)
    g2 = np.asarray(g2, np.float32)
    be1 = np.asarray(be1, np.float32)
    be2 = np.asarray(be2, np.float32)

    # LN(x @ w1.T + b1): a per-feature output bias shifts LN input; it cannot
    # be folded into w1 alone, but appending a constant-1 input feature folds
    # it exactly: [f, 1] @ [w1, b1].T == f @ w1.T + b1.
    if np.any(b1 != 0):
        raise NotImplementedError("nonzero b1 not supported by this kernel")
    if np.any(b2 != 0) or np.any(b3 != 0):
        raise NotImplementedError("nonzero b2/b3 not supported by this kernel")
    if np.any(g1 != 1) or np.any(be1 != 0) or np.any(g2 != 1) or np.any(be2 != 0):
        raise NotImplementedError("non-trivial LN affine not supported by this kernel")

    w1t = np.ascontiguousarray(w1.T)  # [17, 32]
    w2t = np.ascontiguousarray(w2.T)  # [32, 32]
    # w3 rows are (co, ci, f) flattened; permute to [h, (f, co, ci)] fp16
    w3t = np.ascontiguousarray(
        w3.reshape(CH, CH, NF, CH).transpose(3, 2, 0, 1).reshape(CH, NF * RW)
    ).astype(np.float16)
    id32 = np.eye(P, dtype=np.float32)
    id16 = np.eye(P, dtype=np.float16)

    in_maps = []
    for c in range(N_CORES):
        sl = slice(c * EC, (c + 1) * EC)
        in_maps.append(
            {
                "f": np.ascontiguousarray(f[sl]),
                "basis27": np.ascontiguousarray(basis[sl]),
                "w1t": w1t,
                "w2t": w2t,
                "w3t": w3t,
                "ident32": id32,
                "ident16": id16,
            }
        )
    return in_maps


def run(inputs, trace=False, **kw):
    in_maps = prepare_host_inputs(**inputs)
    nc = _get_program(EC)
    res = run_bass_kernel_spmd(nc, in_maps, core_ids=list(range(N_CORES)), trace=trace, **kw)
    out = np.concatenate([r["out"].reshape(EC, 96, 96) for r in res.results], axis=0)
    return out, res


def kernel(**inputs) -> np.ndarray:
    out, _ = run(inputs, trace=False)
    return out


if __name__ == "__main__":
    print("building program...")
    nc = _get_program(EC)
    print("built OK")
